# revision 15
# baseline (speedup 1.0000x reference)
"""Trainium2 Bass kernel for nn_Encoder (dense MLP with stochastic ternarization).

y = tanh(x @ (s1*T(w1,n1)) + b1) @ (s2*T(w2,n2)) + b2,  T(w,n) = (w-n>1) - (w-n<-1)

Sharding: tensor-parallel over the 16384 hidden dim across 8 cores. Each core
gets a 2048-wide hidden shard of w1/noise1/s1/b1 (column-sharded) and the
matching 2048-row shard of w2/noise2; x is replicated (host pre-transposed to
bf16, tiled per 512-batch block). Each core computes partial
yT = (h_shard @ w2_shard).T in fp32 per batch block; per-block, per-dout-half
ReduceScatters(add) hand core c the summed rows {64c..64c+64} of each half,
where s2/b2 are applied. The host reassembles the full output.

Kernel structure (v7):
- Ternary weights in fp8e4 ({-2,0,+2} exact); PE takes fp8 stationary x bf16
  moving. 1 MiB contiguous weight DMA blocks (host pre-tiled).
- Layer 1 of a block pair is interleaved per m-tile, sharing each stationary
  load between the two blocks' matmuls (halves LDWEIGHTS pressure; walrus
  ldw-opt elides the redundant loads).
- Blocks 0/1 interleave layer-1 m-groups at quarter granularity so PE
  consumption tracks the ternarize supply; blocks 2/3 run from resident
  weights.
- h stays in SBUF between layers; each block's partial yT feeds two half-sized
  ReduceScatters that overlap subsequent matmuls. The post-collective path
  (loads + s2*y+b2 + store) runs on sync-DMA + DVE so the ACT queue never
  head-of-line blocks on a collective.

Ternarization: q = w - noise (DVE), tanh(2^30*(q-1)) + tanh(2^30*(q+1)) (ACT)
== (q>1)-(q<-1) doubled; the factor 2 is folded into s1/s2 on the host.
"""

import sys

for _p in ("/opt/trn_rl_repo",):
    if _p not in sys.path:
        sys.path.insert(0, _p)

import numpy as np
import ml_dtypes

import concourse.bass as bass
import concourse.bacc as bacc
import concourse.mybir as mybir
import concourse.tile as tile
import concourse.bass_utils as _bass_utils
from concourse.bass_utils import run_bass_kernel_spmd

# Enable walrus ldw-opt so back-to-back matmuls sharing a stationary operand
# skip the redundant LDWEIGHTS (safe for fp8 weights; the known issue is
# fp32-only). Patches the walrus argv, not the pass list.
if not getattr(_bass_utils, "_ldwopt_patched", False):
    _orig_run_command = _bass_utils.run_command

    def _run_command_ldwopt(argv, **kwargs):
        argv = [
            a
            for a in argv
        ]
        return _orig_run_command(argv, **kwargs)

    _bass_utils.run_command = _run_command_ldwopt
    _bass_utils._ldwopt_patched = True

BF16 = mybir.dt.bfloat16
F32 = mybir.dt.float32
FP8 = mybir.dt.float8e4
NPBF16 = ml_dtypes.bfloat16

N_CORES = 8
B = 2048
DIN = 3072
DHID = 16384
DOUT = 1024
HSH = DHID // N_CORES   # 2048
DSH = DOUT // N_CORES   # 128

K1 = DIN // 128          # 24 contraction tiles, layer 1
KG1 = K1 // 4            # 6 groups of 4 k-tiles (1 MiB weight blocks)
K2 = HSH // 128          # 16 contraction tiles, layer 2
KG2 = K2 // 2            # 8 groups of 2 k2-tiles (1 MiB weight blocks)
NB = B // 512            # 4 batch blocks
MT = HSH // 128          # 16 hidden m-tiles
ND = DOUT // 128         # 8 dout tiles
QW = 512                 # ternarize quarter width
NQ = HSH // QW           # 4 quarters
HROWS = DOUT // 2        # 512 dout rows per RS half
HCHUNK = HROWS // N_CORES  # 64 rows per core per half

BIGK = float(2 ** 30)

TANH = mybir.ActivationFunctionType.Tanh
MULT = mybir.AluOpType.mult
ADD = mybir.AluOpType.add


def build_bass():
    nc = bacc.Bacc("TRN2", target_bir_lowering=False, debug=False, num_devices=N_CORES)

    xtb = nc.dram_tensor("xtb", [NB, 128, K1, 512], BF16, kind="ExternalInput")
    w1g = nc.dram_tensor("w1g", [NQ, KG1, 128, 4, QW], F32, kind="ExternalInput")
    n1g = nc.dram_tensor("n1g", [NQ, KG1, 128, 4, QW], F32, kind="ExternalInput")
    s1h = nc.dram_tensor("s1h", [128, MT], F32, kind="ExternalInput")
    b1m = nc.dram_tensor("b1m", [128, MT], F32, kind="ExternalInput")
    w2g = nc.dram_tensor("w2g", [KG2, 128, 2, DOUT], F32, kind="ExternalInput")
    n2g = nc.dram_tensor("n2g", [KG2, 128, 2, DOUT], F32, kind="ExternalInput")
    s2c = nc.dram_tensor("s2c", [128, 1], F32, kind="ExternalInput")
    b2c = nc.dram_tensor("b2c", [128, 1], F32, kind="ExternalInput")

    # rows 0:64  -> dout 64c .. 64c+64   (half 0)
    # rows 64:128-> dout 512+64c .. +64  (half 1)
    yTc = nc.dram_tensor("yTc", [DSH, B], F32, kind="ExternalOutput")

    with tile.TileContext(nc) as tc:
        with (
            tc.tile_pool(name="const", bufs=1) as cpool,
            tc.tile_pool(name="dram", bufs=1, space="DRAM") as dpool,
            tc.tile_pool(name="t2w1", bufs=KG1) as t2pool,
            tc.tile_pool(name="t2w2", bufs=1) as t22pool,
            tc.tile_pool(name="stage", bufs=2) as spool,
            tc.tile_pool(name="xtn", bufs=2) as xpool,
            tc.tile_pool(name="hblk", bufs=2 * MT) as hpool,
            tc.tile_pool(name="yblk", bufs=4) as ypool,
            tc.tile_pool(name="fin", bufs=2) as fpool,
            tc.tile_pool(name="ps1", bufs=4, space="PSUM") as pspool,
            tc.tile_pool(name="ps2", bufs=4, space="PSUM") as ps2pool,
        ):
            s1_sb = cpool.tile([128, MT], F32, tag="s1")
            b1_sb = cpool.tile([128, MT], F32, tag="b1")
            s2_sb = cpool.tile([128, 1], F32, tag="s2")
            b2_sb = cpool.tile([128, 1], F32, tag="b2")
            nc.scalar.dma_start(s1_sb[:], s1h[:, :])
            nc.scalar.dma_start(b1_sb[:], b1m[:, :])
            nc.scalar.dma_start(s2_sb[:], s2c[:, :])
            nc.scalar.dma_start(b2_sb[:], b2c[:, :])
            kneg = cpool.tile([128, 1], F32, tag="kneg")
            nc.vector.memset(kneg[:], -BIGK)
            kpos = cpool.tile([128, 1], F32, tag="kpos")
            nc.vector.memset(kpos[:], BIGK)

            # per (block, dout-half) partial/scattered buffers
            yT_nh = [[dpool.tile([HROWS, 512], F32, tag=f"yTp{n}{h}",
                                 name=f"yT_n{n}h{h}") for h in range(2)]
                     for n in range(NB)]
            rs_nh = [[dpool.tile([HCHUNK, 512], F32, tag=f"rs{n}{h}",
                                 name=f"rs_n{n}h{h}") for h in range(2)]
                     for n in range(NB)]

            xtn_tiles = {}
            for b in (0, 1):
                xtn_tiles[b] = xpool.tile([128, K1, 512], BF16, tag="xtn",
                                          name=f"xtn{b}")
                nc.scalar.dma_start(xtn_tiles[b][:], xtb[b])

            # ---- ternarize (1 MiB blocks) ----
            t2g = [t2pool.tile([128, 4, HSH], FP8, tag="t2", name=f"t2g_{kg}")
                   for kg in range(KG1)]
            t22 = t22pool.tile([128, K2, DOUT], FP8, tag="t22")

            def tern_block(dst_ap, w_src, n_src, sub_k, fd):
                w_t = spool.tile([128, sub_k, fd], F32, tag="w")
                nc.sync.dma_start(w_t[:], w_src)
                n_t = spool.tile([128, sub_k, fd], F32, tag="n")
                nc.sync.dma_start(n_t[:], n_src)
                nc.vector.tensor_sub(w_t[:], w_t[:], n_t[:])
                a1 = spool.tile([128, sub_k, fd], FP8, tag="a1")
                nc.scalar.activation(a1[:], w_t[:], TANH, bias=kneg[:, 0:1], scale=BIGK)
                a2 = spool.tile([128, sub_k, fd], FP8, tag="a2")
                nc.scalar.activation(a2[:], w_t[:], TANH, bias=kpos[:, 0:1], scale=BIGK)
                nc.vector.tensor_add(dst_ap, a1[:], a2[:])

            for q in range(NQ):
                for kg in range(KG1):
                    tern_block(
                        t2g[kg][:, :, q * QW:(q + 1) * QW],
                        w1g[q, kg], n1g[q, kg], 4, QW,
                    )
            for kg in range(KG2):
                tern_block(
                    t22[:, kg * 2:(kg + 1) * 2, :],
                    w2g[kg], n2g[kg], 2, DOUT,
                )

            # ---- compute ----
            h_sets = {0: [], 1: [], 2: [], 3: []}

            def layer1_mgroup_pair(b0, b1, q):
                for m in range(q * 4, q * 4 + 4):
                    ps0 = pspool.tile([128, 512], F32, tag="ps")
                    ps1t = pspool.tile([128, 512], F32, tag="ps")
                    for k in range(K1):
                        st = t2g[k // 4][:, k % 4, m * 128:(m + 1) * 128]
                        nc.tensor.matmul(ps0[:], st, xtn_tiles[b0][:, k, :],
                                         start=(k == 0), stop=(k == K1 - 1))
                        nc.tensor.matmul(ps1t[:], st, xtn_tiles[b1][:, k, :],
                                         start=(k == 0), stop=(k == K1 - 1))
                    for b, ps in ((b0, ps0), (b1, ps1t)):
                        h_m = hpool.tile([128, 512], BF16, tag="h")
                        nc.scalar.activation(
                            h_m[:], ps[:], TANH,
                            bias=b1_sb[:, m:m + 1], scale=s1_sb[:, m:m + 1],
                        )
                        h_sets[b].append(h_m)

            def rs_half(b, h):
                nc.gpsimd.collective_compute(
                    "ReduceScatter",
                    mybir.AluOpType.add,
                    replica_groups=[list(range(N_CORES))],
                    ins=[yT_nh[b][h].opt()],
                    outs=[rs_nh[b][h].opt()],
                )

            def fin_block(b):
                # post-collective path on sync + DVE only
                rs_sb = fpool.tile([128, 512], F32, tag="rsb")
                nc.sync.dma_start(rs_sb[0:HCHUNK, :], rs_nh[b][0][:, :])
                nc.sync.dma_start(rs_sb[HCHUNK:2 * HCHUNK, :], rs_nh[b][1][:, :])
                out_sb = fpool.tile([128, 512], F32, tag="osb")
                nc.vector.tensor_scalar(
                    out_sb[:], rs_sb[:], s2_sb[:, 0:1], b2_sb[:, 0:1], MULT, ADD,
                )
                nc.sync.dma_start(yTc[:, b * 512:(b + 1) * 512], out_sb[:])

            def layer2_pair(b0, b1):
                for d in range(ND):
                    p0 = ps2pool.tile([128, 512], F32, tag="ps2")
                    p1 = ps2pool.tile([128, 512], F32, tag="ps2")
                    for k2 in range(K2):
                        st = t22[:, k2, d * 128:(d + 1) * 128]
                        nc.tensor.matmul(p0[:], st, h_sets[b0][k2][:],
                                         start=(k2 == 0), stop=(k2 == K2 - 1))
                        nc.tensor.matmul(p1[:], st, h_sets[b1][k2][:],
                                         start=(k2 == 0), stop=(k2 == K2 - 1))
                    for b, p in ((b0, p0), (b1, p1)):
                        y_sb = ypool.tile([128, 512], F32, tag="y")
                        nc.vector.tensor_copy(y_sb[:], p[:])
                        nc.scalar.dma_start(
                            yT_nh[b][d // 4][(d % 4) * 128:(d % 4 + 1) * 128, :],
                            y_sb[:],
                        )
                    if d == 3:
                        rs_half(b0, 0)
                        rs_half(b1, 0)
                rs_half(b0, 1)
                rs_half(b1, 1)
                fin_block(b0)
                fin_block(b1)

            # blocks 0/1 with quarter-interleaved layer-1
            for q in range(NQ):
                layer1_mgroup_pair(0, 1, q)
            layer2_pair(0, 1)

            # blocks 2/3 from resident weights
            for b in (2, 3):
                xtn_tiles[b] = xpool.tile([128, K1, 512], BF16, tag="xtn",
                                          name=f"xtn{b}")
                nc.scalar.dma_start(xtn_tiles[b][:], xtb[b])
            for q in range(NQ):
                layer1_mgroup_pair(2, 3, q)
            layer2_pair(2, 3)

    nc.compile()
    return nc


_NC_CACHE = {}


def _get_nc():
    if "nc" not in _NC_CACHE:
        _NC_CACHE["nc"] = build_bass()
    return _NC_CACHE["nc"]


def _core_rows(c):
    return (slice(HCHUNK * c, HCHUNK * (c + 1)),
            slice(HROWS + HCHUNK * c, HROWS + HCHUNK * (c + 1)))


def _make_in_maps(x, w1, s1, b1, w2, s2, b2, noise1, noise2):
    x = np.asarray(x, dtype=np.float32)
    w1 = np.asarray(w1, dtype=np.float32)
    s1 = np.asarray(s1, dtype=np.float32)
    b1 = np.asarray(b1, dtype=np.float32)
    w2 = np.asarray(w2, dtype=np.float32)
    s2 = np.asarray(s2, dtype=np.float32)
    b2 = np.asarray(b2, dtype=np.float32)
    noise1 = np.asarray(noise1, dtype=np.float32)
    noise2 = np.asarray(noise2, dtype=np.float32)

    xT = x.T.astype(NPBF16)
    xtb = np.ascontiguousarray(xT.reshape(K1, 128, NB, 512).transpose(2, 1, 0, 3))

    def w1_tile(w):   # [din, HSH] -> [NQ, KG1, 128, 4, QW]
        return np.ascontiguousarray(
            w.reshape(KG1, 4, 128, NQ, QW).transpose(3, 0, 2, 1, 4))

    def w2_tile(w):   # [HSH, DOUT] -> [KG2, 128, 2, DOUT]
        return np.ascontiguousarray(
            w.reshape(KG2, 2, 128, DOUT).transpose(0, 2, 1, 3))

    in_maps = []
    for c in range(N_CORES):
        hs = slice(c * HSH, (c + 1) * HSH)
        r0, r1 = _core_rows(c)
        s2rows = np.concatenate([0.5 * s2[r0], 0.5 * s2[r1]])
        b2rows = np.concatenate([b2[r0], b2[r1]])
        in_maps.append({
            "xtb": xtb,
            "w1g": w1_tile(w1[:, hs]),
            "n1g": w1_tile(noise1[:, hs]),
            "s1h": np.ascontiguousarray((0.5 * s1[hs]).reshape(MT, 128).T),
            "b1m": np.ascontiguousarray(b1[hs].reshape(MT, 128).T),
            "w2g": w2_tile(np.ascontiguousarray(w2[hs, :])),
            "n2g": w2_tile(np.ascontiguousarray(noise2[hs, :])),
            "s2c": np.ascontiguousarray(s2rows.reshape(128, 1)),
            "b2c": np.ascontiguousarray(b2rows.reshape(128, 1)),
        })
    return in_maps


def kernel(x, w1, s1, b1, w2, s2, b2, noise1, noise2, _bench_out=None):
    """Full-input, full-output entry point. Shards across 8 NeuronCores."""
    nc = _get_nc()
    in_maps = _make_in_maps(x, w1, s1, b1, w2, s2, b2, noise1, noise2)
    res = run_bass_kernel_spmd(nc, in_maps, core_ids=list(range(N_CORES)))
    if _bench_out is not None:
        _bench_out.append(res)
    yT = np.empty((DOUT, B), dtype=np.float32)
    for c in range(N_CORES):
        r0, r1 = _core_rows(c)
        out_c = res.results[c]["yTc"]
        yT[r0, :] = out_c[0:HCHUNK, :]
        yT[r1, :] = out_c[HCHUNK:2 * HCHUNK, :]
    return np.ascontiguousarray(yT.T).astype(np.float32)


if __name__ == "__main__":
    nc = build_bass()
    print("built OK")
